# revision 1
# baseline (speedup 1.0000x reference)
"""Trainium2 Bass kernel for nn_CudaRenderer.

Per-pixel gather + barycentric weighted sum:
    out[n, d, h, w]  = sum_k baryw[n,h,w,k] * attrs_flat[tri[n,h,w], k, d]   (d < 16)
    out[n, 16, h, w] = tri[n,h,w] != -1
with attrs_flat = attrs.reshape(BZ*NF, 3, 16) and background (tri == -1)
pixels zeroed.

Sharding: data-parallel over the batch axis — each of the 8 NeuronCores
renders one image; the 15.4 MB attrs table is replicated to every core so
the per-pixel gather stays device-local (triangle ids index the *global*
flattened face table, so every core needs the whole table).

Per-core pipeline, tiles of N = 128*G pixels laid out [128 partitions, G]
(pixel = p*G + g):
  1. DMA triangle ids tile, clamp to >= 0 (DVE), compute visibility mask.
  2. indirect_dma_start gathers the 48-float attr row per pixel from HBM.
  3. Mask barycentric weights by visibility, 5 DVE mul/add ops for the
     weighted sum over the 3 vertices.
  4. Strided store to the channel-major output (512 B contiguous runs per
     channel) + visibility plane store.
"""

import numpy as np

import concourse.bacc as bacc
import concourse.bass as bass
import concourse.mybir as mybir
from concourse.tile import TileContext

BZ, NF, D = 8, 10000, 16
H = W = 512
HW = H * W
NFACES = BZ * NF
N_CORES = 8
P = 128

F32 = mybir.dt.float32
I32 = mybir.dt.int32


def renderer_body(tc, outs, ins, *, n_pix, n_faces, G, repeat=1, gather_stride=1,
                  v_split=1, swdge_queues=1, gather_single_packet=False):
    nc = tc.nc
    out = outs["out"]      # [D+1, n_pix] f32
    attrs = ins["attrs"]   # [n_faces, 3*D] f32
    tri = ins["tri"]       # [n_pix] i32
    bary = ins["bary"]     # [n_pix, 3] f32

    N = P * G
    assert n_pix % N == 0
    n_tiles = n_pix // N
    mul = mybir.AluOpType.mult
    add = mybir.AluOpType.add

    with tc.tile_pool(name="pool", bufs=2) as pool:
        for t in [t for _ in range(repeat) for t in range(n_tiles)]:
            sl = slice(t * N, (t + 1) * N)

            tri_t = pool.tile([P, G], I32)
            nc.sync.dma_start(out=tri_t[:], in_=tri[sl].rearrange("(p g) -> p g", g=G))

            idx_t = pool.tile([P, G], I32)
            nc.vector.tensor_scalar_max(idx_t[:], tri_t[:], 0)

            vis_t = pool.tile([P, G], F32)
            nc.vector.tensor_scalar(
                out=vis_t[:], in0=tri_t[:], scalar1=0, scalar2=None,
                op0=mybir.AluOpType.is_ge,
            )

            w_t = pool.tile([P, 3 * G], F32)
            nc.sync.dma_start(
                out=w_t[:], in_=bary[sl, :].rearrange("(p g) k -> p (g k)", g=G)
            )

            # Mask weights by visibility: background pixels get w = 0, so the
            # weighted sum is 0 there with no extra masking op.
            wv_t = pool.tile([P, 3 * G], F32)
            nc.vector.tensor_tensor(
                out=wv_t[:].rearrange("p (g k) -> p g k", k=3),
                in0=w_t[:].rearrange("p (g k) -> p g k", k=3),
                in1=vis_t[:].unsqueeze(2).to_broadcast([P, G, 3]),
                op=mul,
            )

            # Gather the 48-float attr row of each pixel's face. HW semantics
            # of the indirect DMA: one offset per partition, each reading its
            # partition's full dest extent contiguously — so gather 128 rows
            # per call (dest [128, 48], offsets [128, 1]). v_split > 1 spreads
            # consecutive gathers across independent tiles so Tile's per-tile
            # dependency tracking can't chain them on DMA completion.
            R = 3 * D
            Gs = G // v_split
            v_ts = [
                pool.tile([P, Gs * R], F32, tag=f"v{s}", name=f"v{s}")
                for s in range(v_split)
            ]
            for g in range(0, G, gather_stride):
                vt = v_ts[g % v_split]
                gi = nc.gpsimd.indirect_dma_start(
                    out=vt[:, (g // v_split) * R:(g // v_split + 1) * R],
                    out_offset=None,
                    in_=attrs,
                    in_offset=bass.IndirectOffsetOnAxis(ap=idx_t[:, g:g + 1], axis=0),
                )
                if swdge_queues > 1:
                    q = g % swdge_queues
                    gi.ins.queue = f"qPoolDynamic{q or ''}"
                if gather_single_packet:
                    gi.ins.single_packet = True

            # out16/tmp free layout is [d][g] (channel-major within the tile)
            # so the store's innermost dim is contiguous on both sides.
            out16_t = pool.tile([P, G * D], F32)
            tmp_t = pool.tile([P, G * D], F32)
            for s in range(v_split):
                v4 = v_ts[s][:].rearrange("p (g k d) -> p g k d", k=3, d=D)
                w3 = wv_t[:].rearrange("p (g k) -> p g k", k=3)[:, s::v_split, :]
                o3 = out16_t[:].rearrange("p (d g) -> p g d", g=G)[:, s::v_split, :]
                t3 = tmp_t[:].rearrange("p (d g) -> p g d", g=G)[:, s::v_split, :]
                nc.vector.tensor_tensor(
                    out=o3, in0=v4[:, :, 0, :],
                    in1=w3[:, :, 0].unsqueeze(2).to_broadcast([P, Gs, D]), op=mul,
                )
                nc.vector.tensor_tensor(
                    out=t3, in0=v4[:, :, 1, :],
                    in1=w3[:, :, 1].unsqueeze(2).to_broadcast([P, Gs, D]), op=mul,
                )
                nc.vector.tensor_tensor(out=o3, in0=o3, in1=t3, op=add)
                nc.vector.tensor_tensor(
                    out=t3, in0=v4[:, :, 2, :],
                    in1=w3[:, :, 2].unsqueeze(2).to_broadcast([P, Gs, D]), op=mul,
                )
                nc.vector.tensor_tensor(out=o3, in0=o3, in1=t3, op=add)

            # Channel-major store: element order (p, d, g) on both sides;
            # DRAM runs are G contiguous floats per (p, d).
            nc.sync.dma_start(
                out=out[0:D, sl].rearrange("d (p g) -> p d g", g=G),
                in_=out16_t[:].rearrange("p (d g) -> p d g", g=G),
            )
            nc.sync.dma_start(
                out=out[D, sl].rearrange("(p g) -> p g", g=G), in_=vis_t[:]
            )


def build_renderer(n_pix=HW, n_faces=NFACES, G=128, n_cores=N_CORES, repeat=1,
                   gather_stride=1, v_split=4, swdge_queues=1,
                   gather_single_packet=False):
    nc = bacc.Bacc(
        "TRN2",
        target_bir_lowering=False,
        debug=False,
        enable_asserts=False,
        num_devices=n_cores,
        num_swdge_queues=swdge_queues,
    )
    attrs_t = nc.dram_tensor("attrs", [n_faces, 3 * D], F32, kind="ExternalInput")
    tri_t = nc.dram_tensor("tri", [n_pix], I32, kind="ExternalInput")
    bary_t = nc.dram_tensor("bary", [n_pix, 3], F32, kind="ExternalInput")
    out_t = nc.dram_tensor("out", [D + 1, n_pix], F32, kind="ExternalOutput")

    with TileContext(nc) as tc:
        renderer_body(
            tc,
            {"out": out_t.ap()},
            {"attrs": attrs_t.ap(), "tri": tri_t.ap(), "bary": bary_t.ap()},
            n_pix=n_pix,
            n_faces=n_faces,
            G=G,
            repeat=repeat,
            gather_stride=gather_stride,
            v_split=v_split,
            swdge_queues=swdge_queues,
            gather_single_packet=gather_single_packet,
        )
    nc.compile()
    return nc


def make_sharded(nc, n_cores=N_CORES):
    """Non-donating shard_map runner over the 8 axon cores.

    Returns (fn, in_names, out_names, out_avals): fn takes pre-placed global
    (n_cores*dim0, ...) arrays for in_names then zero output buffers, and
    returns concatenated outputs. Mirrors bass2jax.run_bass_via_pjrt but
    reusable/re-callable for timing.
    """
    import jax
    from jax.experimental.shard_map import shard_map
    from jax.sharding import Mesh, PartitionSpec

    from concourse import bass2jax as b2j

    b2j.install_neuronx_cc_hook()
    assert nc.dbg_addr is None and not nc.dbg_callbacks
    partition_name = nc.partition_id_tensor.name if nc.partition_id_tensor else None

    in_names, out_names, out_avals, zero_outs = [], [], [], []
    for alloc in nc.m.functions[0].allocations:
        if not isinstance(alloc, mybir.MemoryLocationSet):
            continue
        name = alloc.memorylocations[0].name
        if alloc.kind == "ExternalInput":
            if name != partition_name:
                in_names.append(name)
        elif alloc.kind == "ExternalOutput":
            shape = tuple(alloc.tensor_shape)
            dtype = mybir.dt.np(alloc.dtype)
            out_names.append(name)
            out_avals.append(jax.core.ShapedArray(shape, dtype))
            zero_outs.append(np.zeros(shape, dtype))
    all_in_names = in_names + out_names
    if partition_name is not None:
        all_in_names = all_in_names + [partition_name]

    def _body(*args):
        operands = list(args)
        if partition_name is not None:
            operands.append(b2j.partition_id_tensor())
        outs = b2j._bass_exec_p.bind(
            *operands,
            out_avals=tuple(out_avals),
            in_names=tuple(all_in_names),
            out_names=tuple(out_names),
            lowering_input_output_aliases=(),
            sim_require_finite=True,
            sim_require_nnan=True,
            nc=nc,
        )
        return tuple(outs)

    devices = jax.devices()[:n_cores]
    mesh = Mesh(np.asarray(devices), ("core",))
    n_args = len(in_names) + len(out_names)
    fn = jax.jit(
        shard_map(
            _body,
            mesh=mesh,
            in_specs=(PartitionSpec("core"),) * n_args,
            out_specs=(PartitionSpec("core"),) * len(out_names),
            check_rep=False,
        ),
        keep_unused=True,
    )
    return fn, in_names, out_names, out_avals, zero_outs, mesh


def make_inputs_concat(attrs, baryw_buffer, triangle_buffer):
    """Concatenated (axis 0) global input arrays keyed by tensor name."""
    attrs_flat = np.ascontiguousarray(
        np.asarray(attrs, dtype=np.float32).reshape(NFACES, 3 * D)
    )
    return {
        "attrs": np.concatenate([attrs_flat] * N_CORES, axis=0),
        "tri": np.ascontiguousarray(
            np.asarray(triangle_buffer, dtype=np.int32).reshape(N_CORES * HW)
        ),
        "bary": np.ascontiguousarray(
            np.asarray(baryw_buffer, dtype=np.float32).reshape(N_CORES * HW, 3)
        ),
    }


_CACHED = {}


def _get_nc(**build_kwargs):
    key = tuple(sorted(build_kwargs.items()))
    if key not in _CACHED:
        _CACHED[key] = build_renderer(**build_kwargs)
    return _CACHED[key]


def run(attrs, baryw_buffer, triangle_buffer, trace=False, **run_kwargs):
    """Shard, run on 8 cores, gather. Returns (output, BassKernelResults)."""
    from concourse import bass_utils

    nc = _get_nc()
    attrs_flat = np.ascontiguousarray(
        np.asarray(attrs, dtype=np.float32).reshape(NFACES, 3 * D)
    )
    in_maps = []
    for c in range(N_CORES):
        in_maps.append(
            {
                "attrs": attrs_flat,
                "tri": np.ascontiguousarray(
                    np.asarray(triangle_buffer[c], dtype=np.int32).reshape(HW)
                ),
                "bary": np.ascontiguousarray(
                    np.asarray(baryw_buffer[c], dtype=np.float32).reshape(HW, 3)
                ),
            }
        )
    br = bass_utils.run_bass_kernel_spmd(
        nc, in_maps, list(range(N_CORES)), trace=trace, **run_kwargs
    )
    out = np.stack(
        [np.asarray(br.results[c]["out"]).reshape(D + 1, H, W) for c in range(N_CORES)]
    )
    return out, br


def kernel(attrs, baryw_buffer, triangle_buffer):
    out, _ = run(attrs, baryw_buffer, triangle_buffer)
    return out

